# revision 1
# baseline (speedup 1.0000x reference)
"""MixTreeLSTMCell Trainium2 kernel (8 NeuronCores, SPMD).

Strategy
--------
The cell evaluates one of two branches per node depending on t in {0,1}.
Computing both branches for every node doubles the matmul flops and makes
the kernel PE-bound (~2x the memory roofline).  Instead the host
partitions the nodes by type and hands every core an equal number of
type-0 and type-1 nodes (padded up to a 512-node tile multiple), so the
device program has two static segments and no per-node select.

All matmul operands are laid out feature-major on the host (x^T, h^T and
the transposed weight matrices) so the device does no on-chip transposes,
and are cast to fp16 (halves the HBM traffic at ~2e-3 relative error;
matmuls accumulate fp32 in PSUM).  The host appends a ones-row to x^T and
the iou bias rows to the transposed W matrices, folding the iou bias into
the matmul; the f-gate biases are applied natively per partition by the
scalar engine when it drains PSUM.  The vector engine runs the remaining
elementwise chain in fp32.  Loads are issued in 2048-node macro tiles for
4 KiB-per-partition DMA runs, split across both HWDGE rings (sync/scalar)
with stores on SWDGE (gpsimd) for queue parallelism.  Outputs are
produced feature-major fp32 and un-permuted/transposed on the host.
"""

from contextlib import ExitStack

import numpy as np

import concourse.bacc as bacc
import concourse.tile as tile
from concourse import mybir
from concourse import bass_utils

F32 = mybir.dt.float32
FP16 = mybir.dt.float16
NP_FP16 = np.float16

N_NODES = 131072
X = 300
XP = X + 1            # x rows + folded-bias ones row
XK2 = XP - 256        # rows in the third (partial) x k-tile
H = 256
CORES = 8
TILE_N = 512          # nodes per compute tile (matmul free dim)
MACRO = 4 * TILE_N    # nodes per DMA macro tile

# Set by test harness to profile; LAST_EXEC_NS is filled after each run.
TRACE = False
LAST_EXEC_NS = None

_PROGRAM_CACHE = {}


def _round_up(v, m):
    return (v + m - 1) // m * m


def _build_program(T0, T1):
    """Trace + compile the SPMD program for T0 type-0 tiles and T1 type-1
    tiles of 512 nodes each (identical on all cores)."""
    key = (T0, T1)
    if key in _PROGRAM_CACHE:
        return _PROGRAM_CACHE[key]

    Nc = (T0 + T1) * TILE_N
    nc = bacc.Bacc("TRN2", target_bir_lowering=False, debug=False)

    xT = nc.dram_tensor("xT", [XP, Nc], FP16, kind="ExternalInput").ap()
    hT = nc.dram_tensor("hT", [2 * H, Nc], FP16, kind="ExternalInput").ap()
    cT = nc.dram_tensor("cT", [2 * H, Nc], FP16, kind="ExternalInput").ap()

    WnT = nc.dram_tensor("WnT", [XP, 3 * H], FP16, kind="ExternalInput").ap()
    UnT = nc.dram_tensor("UnT", [2 * H, 3 * H], FP16, kind="ExternalInput").ap()
    UfwT = nc.dram_tensor("UfwT", [2 * H, 2 * H], FP16, kind="ExternalInput").ap()
    WsT = nc.dram_tensor("WsT", [XP, 3 * H], FP16, kind="ExternalInput").ap()
    UsT = nc.dram_tensor("UsT", [H, 3 * H], FP16, kind="ExternalInput").ap()
    UfswT = nc.dram_tensor("UfswT", [H, H], FP16, kind="ExternalInput").ap()

    bias_fn = nc.dram_tensor("bias_fn", [128, 4], F32, kind="ExternalInput").ap()
    bias_fs = nc.dram_tensor("bias_fs", [128, 2], F32, kind="ExternalInput").ap()

    hOT = nc.dram_tensor("hOT", [H, Nc], F32, kind="ExternalOutput").ap()
    cOT = nc.dram_tensor("cOT", [H, Nc], F32, kind="ExternalOutput").ap()

    # feature-major [p, ko, n] views of the DRAM activations
    hT_v = hT.rearrange("(ko p) n -> p ko n", p=128)
    cT_v = cT.rearrange("(ko p) n -> p ko n", p=128)
    hOT_v = hOT.rearrange("(ko p) n -> p ko n", p=128)
    cOT_v = cOT.rearrange("(ko p) n -> p ko n", p=128)
    UnT_v = UnT.rearrange("(ko p) m -> p ko m", p=128)
    UfwT_v = UfwT.rearrange("(ko p) m -> p ko m", p=128)
    UsT_v = UsT.rearrange("(ko p) m -> p ko m", p=128)
    UfswT_v = UfswT.rearrange("(ko p) m -> p ko m", p=128)

    SIG = mybir.ActivationFunctionType.Sigmoid
    TANH = mybir.ActivationFunctionType.Tanh

    with tile.TileContext(nc) as tc, ExitStack() as stack:
        wp = stack.enter_context(tc.tile_pool(name="w", bufs=1))
        io = stack.enter_context(tc.tile_pool(name="io", bufs=3))
        mid = stack.enter_context(tc.tile_pool(name="mid", bufs=2))
        psf = stack.enter_context(tc.tile_pool(name="psf", bufs=4, space="PSUM"))
        ps2 = stack.enter_context(tc.tile_pool(name="ps2", bufs=2, space="PSUM"))

        # --- resident weights; f-gate weights first (tile 0 needs them
        # before anything else), the big iou weights via the scalar ring so
        # they don't delay the first macro's activation loads ---
        Ufw_sb = wp.tile([128, 4, 2 * H], FP16)
        nc.sync.dma_start(out=Ufw_sb, in_=UfwT_v)
        Ufsw_sb = wp.tile([128, 2, H], FP16)
        nc.sync.dma_start(out=Ufsw_sb, in_=UfswT_v)
        bfn_sb = wp.tile([128, 4], F32)
        nc.sync.dma_start(out=bfn_sb, in_=bias_fn)
        bfs_sb = wp.tile([128, 2], F32)
        nc.sync.dma_start(out=bfs_sb, in_=bias_fs)
        Wn_sb = wp.tile([128, 3, 3 * H], FP16)
        Ws_sb = wp.tile([128, 3, 3 * H], FP16)
        for k in range(2):
            nc.scalar.dma_start(out=Wn_sb[:, k, :], in_=WnT[128 * k : 128 * (k + 1), :])
            nc.scalar.dma_start(out=Ws_sb[:, k, :], in_=WsT[128 * k : 128 * (k + 1), :])
        nc.scalar.dma_start(out=Wn_sb[:XK2, 2, :], in_=WnT[256:XP, :])
        nc.scalar.dma_start(out=Ws_sb[:XK2, 2, :], in_=WsT[256:XP, :])
        Un_sb = wp.tile([128, 4, 3 * H], FP16)
        nc.scalar.dma_start(out=Un_sb, in_=UnT_v)
        Us_sb = wp.tile([128, 2, 3 * H], FP16)
        nc.scalar.dma_start(out=Us_sb, in_=UsT_v)

        def iou_mm(ps, xt, ht, htild, j, W_sb, U_sb, uk, m, ncol):
            """All matmuls accumulating iou m-tile m into ps[:, :ncol]."""
            ms = slice(128 * m, 128 * (m + 1))
            ns = slice(j * TILE_N, j * TILE_N + ncol)
            nc.tensor.matmul(ps, W_sb[:, 0, ms], xt[:, 0, ns], start=True, stop=False)
            nc.tensor.matmul(ps, W_sb[:, 1, ms], xt[:, 1, ns], start=False, stop=False)
            nc.tensor.matmul(
                ps, W_sb[:XK2, 2, ms], xt[:XK2, 2, ns], start=False, stop=False
            )
            for k in range(uk):
                rhs = ht[:, k, ns] if htild is None else htild[:, k, :]
                nc.tensor.matmul(
                    ps, U_sb[:, k, ms], rhs, start=False, stop=(k == uk - 1)
                )

        def do_tile(br, xt, ht, ct, j, n0, ncol):
            """Process one <=512-node tile; xt/ht/ct are MACRO tiles, j the
            tile index inside the macro, n0 the DRAM node offset."""
            ns = slice(j * TILE_N, j * TILE_N + ncol)

            # --- forget gates f: [128, 4, ncol] = 512 features x nodes ---
            f_full = mid.tile([128, 4, TILE_N], FP16, tag="f", name="f")
            f = f_full[:, :, :ncol]
            if br == 0:
                for m in range(4):
                    ps_full = psf.tile([128, TILE_N], F32, tag="psf", name="ps")
                    ps = ps_full[:, :ncol]
                    for k in range(4):
                        nc.tensor.matmul(
                            ps,
                            Ufw_sb[:, k, 128 * m : 128 * (m + 1)],
                            ht[:, k, ns],
                            start=(k == 0),
                            stop=(k == 3),
                        )
                    nc.scalar.activation(
                        out=f[:, m, :], in_=ps, func=SIG, bias=bfn_sb[:, m : m + 1]
                    )
            else:
                for child in range(2):
                    for m in range(2):
                        ps_full = psf.tile([128, TILE_N], F32, tag="psf", name="ps")
                        ps = ps_full[:, :ncol]
                        for k in range(2):
                            nc.tensor.matmul(
                                ps,
                                Ufsw_sb[:, k, 128 * m : 128 * (m + 1)],
                                ht[:, 2 * child + k, ns],
                                start=(k == 0),
                                stop=(k == 1),
                            )
                        nc.scalar.activation(
                            out=f[:, 2 * child + m, :],
                            in_=ps,
                            func=SIG,
                            bias=bfs_sb[:, m : m + 1],
                        )

            # prod = f * c_child (in place), c_red = child0 + child1
            nc.vector.tensor_mul(out=f, in0=f, in1=ct[:, :, ns])
            cred_full = mid.tile([128, 2, TILE_N], F32, tag="cred", name="cred")
            cred = cred_full[:, :, :ncol]
            nc.vector.tensor_add(out=cred, in0=f[:, 0:2, :], in1=f[:, 2:4, :])


            htild = None
            if br == 1:
                htild_full = mid.tile([128, 2, TILE_N], FP16, tag="htild", name="htild")
                htild = htild_full[:, :, :ncol]
                nc.vector.tensor_add(out=htild, in0=ht[:, 0:2, ns], in1=ht[:, 2:4, ns])

            # --- iou gates: 3 m-pairs, each a 2-bank PSUM + single ACT ---
            # (iou bias is folded into the matmul via the x^T ones row)
            gates_full = mid.tile([128, 6, TILE_N], FP16, tag="gates", name="gates")
            gates = gates_full[:, :, :ncol]
            for mp in range(3):
                ps_full = ps2.tile([128, 2, TILE_N], F32, tag="ps2", name="ps")
                ps = ps_full[:, :, :ncol]
                for m2 in range(2):
                    m = 2 * mp + m2
                    if br == 0:
                        iou_mm(ps[:, m2, :], xt, ht, None, j, Wn_sb, Un_sb, 4, m, ncol)
                    else:
                        iou_mm(ps[:, m2, :], xt, ht, htild, j, Ws_sb, Us_sb, 2, m, ncol)
                nc.scalar.activation(
                    out=gates[:, 2 * mp : 2 * mp + 2, :],
                    in_=ps,
                    func=TANH if mp == 2 else SIG,
                )

            # c = sig(i)*tanh(u) + c_red ; h = sig(o)*tanh(c)
            cout_full = mid.tile([128, 2, TILE_N], F32, tag="cout", name="cout")
            cout = cout_full[:, :, :ncol]
            nc.vector.tensor_mul(out=cout, in0=gates[:, 0:2, :], in1=gates[:, 4:6, :])
            nc.vector.tensor_add(out=cout, in0=cout, in1=cred)
            tct_full = mid.tile([128, 2, TILE_N], FP16, tag="tct", name="tct")
            tct = tct_full[:, :, :ncol]
            nc.scalar.activation(out=tct, in_=cout, func=TANH)
            hout_full = mid.tile([128, 2, TILE_N], F32, tag="hout", name="hout")
            hout = hout_full[:, :, :ncol]
            nc.vector.tensor_mul(out=hout, in0=gates[:, 2:4, :], in1=tct)

            nc.gpsimd.dma_start(out=hOT_v[:, :, n0 : n0 + ncol], in_=hout)
            nc.gpsimd.dma_start(out=cOT_v[:, :, n0 : n0 + ncol], in_=cout)

        # macro-tile loop: load up to 2048 nodes at a time, compute 4 tiles.
        # The first two macros are single tiles so the PE ramps up as soon
        # as the first 512-node slice lands instead of waiting for 2048.
        segs = [(0, 0, T0), (1, T0 * TILE_N, T1)]
        first = True
        for br, base, T in segs:
            starts = []
            g = 0
            while g < T:
                nt = 1 if (first and g < 2) else min(4, T - g)
                starts.append((g, nt))
                g += nt
            first = False
            for g, nt in starts:
                n0 = base + g * TILE_N
                w = nt * TILE_N
                xt_full = io.tile([128, 3, MACRO], FP16, tag="xt", name="xt")
                xt = xt_full[:, :, :w]
                for k in range(2):
                    nc.sync.dma_start(
                        out=xt[:, k, :], in_=xT[128 * k : 128 * (k + 1), n0 : n0 + w]
                    )
                nc.sync.dma_start(out=xt[:XK2, 2, :], in_=xT[256:XP, n0 : n0 + w])
                ht_full = io.tile([128, 4, MACRO], FP16, tag="ht", name="ht")
                ht = ht_full[:, :, :w]
                nc.sync.dma_start(out=ht, in_=hT_v[:, :, n0 : n0 + w])
                ct_full = io.tile([128, 4, MACRO], FP16, tag="ct", name="ct")
                ct = ct_full[:, :, :w]
                nc.scalar.dma_start(out=ct, in_=cT_v[:, :, n0 : n0 + w])
                for j in range(nt):
                    do_tile(br, xt, ht, ct, j, n0 + j * TILE_N, TILE_N)

    nc.compile()
    _PROGRAM_CACHE[key] = nc
    return nc


def kernel(x, h_child, c_child, t, W_iou, U_iou, b_iou, U_f_w, U_f_b,
           W_iou_s, U_iou_s, b_iou_s, U_f_s_w, U_f_s_b):
    global LAST_EXEC_NS
    x = np.asarray(x, dtype=np.float32)
    h_child = np.asarray(h_child, dtype=np.float32)
    c_child = np.asarray(c_child, dtype=np.float32)
    t = np.asarray(t)
    n = x.shape[0]

    # --- host partition: equal per-core type counts, padded to tiles ---
    idx0 = np.flatnonzero(t == 0)
    idx1 = np.flatnonzero(t != 0)
    n0, n1 = len(idx0), len(idx1)

    def pad_split(idx, cnt):
        if cnt == 0:
            return np.zeros((CORES, 0), dtype=np.int64), 0
        per = _round_up(-(-cnt // CORES), TILE_N)
        padded = np.concatenate(
            [idx, np.full(CORES * per - cnt, idx[-1], dtype=idx.dtype)]
        )
        return padded.reshape(CORES, per).astype(np.int64), per

    chunks0, P0 = pad_split(idx0, n0)
    chunks1, P1 = pad_split(idx1, n1)
    T0, T1 = P0 // TILE_N, P1 // TILE_N

    nc = _build_program(T0, T1)

    # --- weights (shared across cores) ---
    hc2 = h_child.reshape(n, 2 * H)
    cc2 = c_child.reshape(n, 2 * H)

    def bias_tile(v, m):
        # [m*128] bias vector -> [128, m] per-partition layout
        return np.ascontiguousarray(
            np.asarray(v, np.float32).reshape(-1)[: 128 * m].reshape(m, 128).T
        )

    def w_with_bias(W, b):
        # [XP, 768] = W^T with the iou bias as the trailing row
        return np.concatenate(
            [np.asarray(W, np.float32).T, np.asarray(b, np.float32).reshape(1, -1)]
        ).astype(NP_FP16)

    wmap = {
        "WnT": w_with_bias(W_iou, b_iou),
        "UnT": np.ascontiguousarray(np.asarray(U_iou, np.float32).T).astype(NP_FP16),
        "UfwT": np.ascontiguousarray(np.asarray(U_f_w, np.float32).T).astype(NP_FP16),
        "WsT": w_with_bias(W_iou_s, b_iou_s),
        "UsT": np.ascontiguousarray(np.asarray(U_iou_s, np.float32).T).astype(NP_FP16),
        "UfswT": np.ascontiguousarray(np.asarray(U_f_s_w, np.float32).T).astype(NP_FP16),
        "bias_fn": bias_tile(U_f_b, 4),
        "bias_fs": bias_tile(U_f_s_b, 2),
    }

    in_maps = []
    ones = None
    for i in range(CORES):
        I = np.concatenate([chunks0[i], chunks1[i]])
        if ones is None:
            ones = np.ones((1, len(I)), dtype=NP_FP16)
        m = dict(wmap)
        m["xT"] = np.concatenate([x[I].T.astype(NP_FP16), ones])
        m["hT"] = hc2[I].T.astype(NP_FP16)
        m["cT"] = cc2[I].T.astype(NP_FP16)
        in_maps.append(m)

    res = bass_utils.run_bass_kernel_spmd(
        nc, in_maps, core_ids=list(range(CORES)), trace=TRACE
    )
    LAST_EXEC_NS = res.exec_time_ns

    # --- scatter back ---
    h_out = np.empty((n, H), dtype=np.float32)
    c_out = np.empty((n, H), dtype=np.float32)
    if n0:
        h0 = np.concatenate([res.results[i]["hOT"][:, :P0].T for i in range(CORES)])
        c0 = np.concatenate([res.results[i]["cOT"][:, :P0].T for i in range(CORES)])
        h_out[idx0] = h0[:n0]
        c_out[idx0] = c0[:n0]
    if n1:
        h1 = np.concatenate([res.results[i]["hOT"][:, P0:].T for i in range(CORES)])
        c1 = np.concatenate([res.results[i]["cOT"][:, P0:].T for i in range(CORES)])
        h_out[idx1] = h1[:n1]
        c_out[idx1] = c1[:n1]
    return h_out, c_out



# revision 2
# speedup vs baseline: 1.1301x; 1.1301x over previous
"""MixTreeLSTMCell Trainium2 kernel (8 NeuronCores, SPMD).

Strategy
--------
The cell evaluates one of two branches per node depending on t in {0,1}.
The host partitions the nodes by type and hands every core an equal
number of type-0 and type-1 nodes (padded up to 512-node tiles), so the
device program has two static segments and no per-node select.

All matmul operands are laid out feature-major on the host (x^T, h^T and
the transposed weight matrices) and cast to fp16 (matmuls accumulate
fp32 in PSUM).  The x/W contraction dim is zero-padded from 301 (300
features + folded-bias ones row) to 384 so every matmul is a uniform
K=128 tile: a K=45 matmul forces a 64-row PE-array reconfig that costs
~+100 ns on itself AND on its successor (measured), so uniform K=128
keeps the whole stream at the 216 ns/matmul roofline.

The iou bias rows are folded into the matmul via the ones row; f-gate
biases are applied per partition by the scalar engine when it drains
PSUM.  The vector engine runs the elementwise chain in fp16 (2x DVE
rate).  Outputs are stored fp16 feature-major and un-permuted/cast on
the host.  Loads are issued in 2048-node macro tiles for 4 KiB-per-
partition DMA runs, split across both HWDGE rings (sync/scalar) with
stores on SWDGE (gpsimd).
"""

from contextlib import ExitStack

import numpy as np

import concourse.bacc as bacc
import concourse.tile as tile
from concourse import mybir
from concourse import bass_utils

F32 = mybir.dt.float32
FP16 = mybir.dt.float16
NP_FP16 = np.float16

N_NODES = 131072
X = 300
XP = X + 1            # x rows + folded-bias ones row
XPAD = 384            # padded to 3 full K=128 tiles (pad rows zero)
H = 256
CORES = 8
TILE_N = 512          # nodes per compute tile (matmul free dim)
MACRO = 4 * TILE_N    # nodes per DMA macro tile

# Set by test harness to profile; LAST_EXEC_NS is filled after each run.
TRACE = False
LAST_EXEC_NS = None

_PROGRAM_CACHE = {}


def _round_up(v, m):
    return (v + m - 1) // m * m


def _build_program(T0, T1):
    """Trace + compile the SPMD program for T0 type-0 tiles and T1 type-1
    tiles of 512 nodes each (identical on all cores)."""
    key = (T0, T1)
    if key in _PROGRAM_CACHE:
        return _PROGRAM_CACHE[key]

    Nc = (T0 + T1) * TILE_N
    nc = bacc.Bacc("TRN2", target_bir_lowering=False, debug=False)

    xT = nc.dram_tensor("xT", [XPAD, Nc], FP16, kind="ExternalInput").ap()
    hT = nc.dram_tensor("hT", [2 * H, Nc], FP16, kind="ExternalInput").ap()
    cT = nc.dram_tensor("cT", [2 * H, Nc], FP16, kind="ExternalInput").ap()

    WnT = nc.dram_tensor("WnT", [XPAD, 3 * H], FP16, kind="ExternalInput").ap()
    UnT = nc.dram_tensor("UnT", [2 * H, 3 * H], FP16, kind="ExternalInput").ap()
    UfwT = nc.dram_tensor("UfwT", [2 * H, 2 * H], FP16, kind="ExternalInput").ap()
    WsT = nc.dram_tensor("WsT", [XPAD, 3 * H], FP16, kind="ExternalInput").ap()
    UsT = nc.dram_tensor("UsT", [H, 3 * H], FP16, kind="ExternalInput").ap()
    UfswT = nc.dram_tensor("UfswT", [H, H], FP16, kind="ExternalInput").ap()

    bias_fn = nc.dram_tensor("bias_fn", [128, 4], F32, kind="ExternalInput").ap()
    bias_fs = nc.dram_tensor("bias_fs", [128, 2], F32, kind="ExternalInput").ap()

    hOT = nc.dram_tensor("hOT", [H, Nc], FP16, kind="ExternalOutput").ap()
    cOT = nc.dram_tensor("cOT", [H, Nc], FP16, kind="ExternalOutput").ap()

    # feature-major [p, ko, n] views of the DRAM activations
    xT_v = xT.rearrange("(ko p) n -> p ko n", p=128)
    hT_v = hT.rearrange("(ko p) n -> p ko n", p=128)
    cT_v = cT.rearrange("(ko p) n -> p ko n", p=128)
    hOT_v = hOT.rearrange("(ko p) n -> p ko n", p=128)
    cOT_v = cOT.rearrange("(ko p) n -> p ko n", p=128)
    WnT_v = WnT.rearrange("(ko p) m -> p ko m", p=128)
    WsT_v = WsT.rearrange("(ko p) m -> p ko m", p=128)
    UnT_v = UnT.rearrange("(ko p) m -> p ko m", p=128)
    UfwT_v = UfwT.rearrange("(ko p) m -> p ko m", p=128)
    UsT_v = UsT.rearrange("(ko p) m -> p ko m", p=128)
    UfswT_v = UfswT.rearrange("(ko p) m -> p ko m", p=128)

    SIG = mybir.ActivationFunctionType.Sigmoid
    TANH = mybir.ActivationFunctionType.Tanh

    with tile.TileContext(nc) as tc, ExitStack() as stack:
        wp = stack.enter_context(tc.tile_pool(name="w", bufs=1))
        io = stack.enter_context(tc.tile_pool(name="io", bufs=3))
        mid = stack.enter_context(tc.tile_pool(name="mid", bufs=2))
        psf = stack.enter_context(tc.tile_pool(name="psf", bufs=4, space="PSUM"))
        ps2 = stack.enter_context(tc.tile_pool(name="ps2", bufs=2, space="PSUM"))

        # --- resident weights; f-gate weights first (tile 0 needs them
        # before anything else), split per k-slice so descriptors spread
        # over several DMA engines; the big iou weights go via the scalar
        # ring so they don't delay the first macro's activation loads ---
        Ufw_sb = wp.tile([128, 4, 2 * H], FP16)
        for k in range(4):
            nc.sync.dma_start(out=Ufw_sb[:, k, :], in_=UfwT_v[:, k, :])
        Ufsw_sb = wp.tile([128, 2, H], FP16)
        nc.scalar.dma_start(out=Ufsw_sb, in_=UfswT_v)
        bfn_sb = wp.tile([128, 4], F32)
        nc.scalar.dma_start(out=bfn_sb, in_=bias_fn)
        bfs_sb = wp.tile([128, 2], F32)
        nc.scalar.dma_start(out=bfs_sb, in_=bias_fs)
        Wn_sb = wp.tile([128, 3, 3 * H], FP16)
        Ws_sb = wp.tile([128, 3, 3 * H], FP16)
        for k in range(3):
            nc.scalar.dma_start(out=Wn_sb[:, k, :], in_=WnT_v[:, k, :])
        for k in range(3):
            nc.scalar.dma_start(out=Ws_sb[:, k, :], in_=WsT_v[:, k, :])
        Un_sb = wp.tile([128, 4, 3 * H], FP16)
        for k in range(4):
            nc.scalar.dma_start(out=Un_sb[:, k, :], in_=UnT_v[:, k, :])
        Us_sb = wp.tile([128, 2, 3 * H], FP16)
        nc.scalar.dma_start(out=Us_sb, in_=UsT_v)

        def iou_mm(ps, xt, ht, htild, j, W_sb, U_sb, uk, m, ncol):
            """All matmuls accumulating iou m-tile m into ps[:, :ncol]."""
            ms = slice(128 * m, 128 * (m + 1))
            ns = slice(j * TILE_N, j * TILE_N + ncol)
            for k in range(3):
                nc.tensor.matmul(
                    ps, W_sb[:, k, ms], xt[:, k, ns], start=(k == 0), stop=False
                )
            for k in range(uk):
                rhs = ht[:, k, ns] if htild is None else htild[:, k, :]
                nc.tensor.matmul(
                    ps, U_sb[:, k, ms], rhs, start=False, stop=(k == uk - 1)
                )

        def do_tile(br, xt, ht, ct, j, n0, ncol):
            """Process one <=512-node tile; xt/ht/ct are MACRO tiles, j the
            tile index inside the macro, n0 the DRAM node offset."""
            ns = slice(j * TILE_N, j * TILE_N + ncol)

            # --- forget gates f: [128, 4, ncol] = 512 features x nodes ---
            f_full = mid.tile([128, 4, TILE_N], FP16, tag="f", name="f")
            f = f_full[:, :, :ncol]
            if br == 0:
                for m in range(4):
                    ps_full = psf.tile([128, TILE_N], F32, tag="psf", name="ps")
                    ps = ps_full[:, :ncol]
                    for k in range(4):
                        nc.tensor.matmul(
                            ps,
                            Ufw_sb[:, k, 128 * m : 128 * (m + 1)],
                            ht[:, k, ns],
                            start=(k == 0),
                            stop=(k == 3),
                        )
                    nc.scalar.activation(
                        out=f[:, m, :], in_=ps, func=SIG, bias=bfn_sb[:, m : m + 1]
                    )
            else:
                for child in range(2):
                    for m in range(2):
                        ps_full = psf.tile([128, TILE_N], F32, tag="psf", name="ps")
                        ps = ps_full[:, :ncol]
                        for k in range(2):
                            nc.tensor.matmul(
                                ps,
                                Ufsw_sb[:, k, 128 * m : 128 * (m + 1)],
                                ht[:, 2 * child + k, ns],
                                start=(k == 0),
                                stop=(k == 1),
                            )
                        nc.scalar.activation(
                            out=f[:, 2 * child + m, :],
                            in_=ps,
                            func=SIG,
                            bias=bfs_sb[:, m : m + 1],
                        )

            # prod = f * c_child (in place), c_red = child0 + child1
            nc.vector.tensor_mul(out=f, in0=f, in1=ct[:, :, ns])
            cred_full = mid.tile([128, 2, TILE_N], FP16, tag="cred", name="cred")
            cred = cred_full[:, :, :ncol]
            nc.vector.tensor_add(out=cred, in0=f[:, 0:2, :], in1=f[:, 2:4, :])

            htild = None
            if br == 1:
                htild_full = mid.tile([128, 2, TILE_N], FP16, tag="htild", name="htild")
                htild = htild_full[:, :, :ncol]
                nc.vector.tensor_add(out=htild, in0=ht[:, 0:2, ns], in1=ht[:, 2:4, ns])

            # --- iou gates: 3 m-pairs, each a 2-bank PSUM + single ACT ---
            # (iou bias is folded into the matmul via the x^T ones row)
            gates_full = mid.tile([128, 6, TILE_N], FP16, tag="gates", name="gates")
            gates = gates_full[:, :, :ncol]
            for mp in range(3):
                ps_full = ps2.tile([128, 2, TILE_N], F32, tag="ps2", name="ps")
                ps = ps_full[:, :, :ncol]
                for m2 in range(2):
                    m = 2 * mp + m2
                    if br == 0:
                        iou_mm(ps[:, m2, :], xt, ht, None, j, Wn_sb, Un_sb, 4, m, ncol)
                    else:
                        iou_mm(ps[:, m2, :], xt, ht, htild, j, Ws_sb, Us_sb, 2, m, ncol)
                nc.scalar.activation(
                    out=gates[:, 2 * mp : 2 * mp + 2, :],
                    in_=ps,
                    func=TANH if mp == 2 else SIG,
                )

            # c = sig(i)*tanh(u) + c_red ; h = sig(o)*tanh(c)
            cout_full = mid.tile([128, 2, TILE_N], FP16, tag="cout", name="cout")
            cout = cout_full[:, :, :ncol]
            nc.vector.tensor_mul(out=cout, in0=gates[:, 0:2, :], in1=gates[:, 4:6, :])
            nc.vector.tensor_add(out=cout, in0=cout, in1=cred)
            tct_full = mid.tile([128, 2, TILE_N], FP16, tag="tct", name="tct")
            tct = tct_full[:, :, :ncol]
            nc.scalar.activation(out=tct, in_=cout, func=TANH)
            hout_full = mid.tile([128, 2, TILE_N], FP16, tag="hout", name="hout")
            hout = hout_full[:, :, :ncol]
            nc.vector.tensor_mul(out=hout, in0=gates[:, 2:4, :], in1=tct)

            nc.gpsimd.dma_start(out=hOT_v[:, :, n0 : n0 + ncol], in_=hout)
            nc.gpsimd.dma_start(out=cOT_v[:, :, n0 : n0 + ncol], in_=cout)

        # macro-tile loop: load up to 2048 nodes at a time, compute 4 tiles.
        # The first two macros are single tiles so the PE ramps up as soon
        # as the first 512-node slice lands instead of waiting for 2048.
        segs = [(0, 0, T0), (1, T0 * TILE_N, T1)]
        first = True
        for br, base, T in segs:
            starts = []
            g = 0
            while g < T:
                nt = 1 if (first and g < 2) else min(4, T - g)
                starts.append((g, nt))
                g += nt
            first = False
            for g, nt in starts:
                n0 = base + g * TILE_N
                w = nt * TILE_N
                ht_full = io.tile([128, 4, MACRO], FP16, tag="ht", name="ht")
                ht = ht_full[:, :, :w]
                nc.sync.dma_start(out=ht, in_=hT_v[:, :, n0 : n0 + w])
                xt_full = io.tile([128, 3, MACRO], FP16, tag="xt", name="xt")
                xt = xt_full[:, :, :w]
                nc.sync.dma_start(out=xt, in_=xT_v[:, :, n0 : n0 + w])
                ct_full = io.tile([128, 4, MACRO], FP16, tag="ct", name="ct")
                ct = ct_full[:, :, :w]
                nc.scalar.dma_start(out=ct, in_=cT_v[:, :, n0 : n0 + w])
                for j in range(nt):
                    do_tile(br, xt, ht, ct, j, n0 + j * TILE_N, TILE_N)

    nc.compile()
    _PROGRAM_CACHE[key] = nc
    return nc


def kernel(x, h_child, c_child, t, W_iou, U_iou, b_iou, U_f_w, U_f_b,
           W_iou_s, U_iou_s, b_iou_s, U_f_s_w, U_f_s_b):
    global LAST_EXEC_NS
    x = np.asarray(x, dtype=np.float32)
    h_child = np.asarray(h_child, dtype=np.float32)
    c_child = np.asarray(c_child, dtype=np.float32)
    t = np.asarray(t)
    n = x.shape[0]

    # --- host partition: equal per-core type counts, padded to tiles ---
    idx0 = np.flatnonzero(t == 0)
    idx1 = np.flatnonzero(t != 0)
    n0, n1 = len(idx0), len(idx1)

    def pad_split(idx, cnt):
        if cnt == 0:
            return np.zeros((CORES, 0), dtype=np.int64), 0
        per = _round_up(-(-cnt // CORES), TILE_N)
        padded = np.concatenate(
            [idx, np.full(CORES * per - cnt, idx[-1], dtype=idx.dtype)]
        )
        return padded.reshape(CORES, per).astype(np.int64), per

    chunks0, P0 = pad_split(idx0, n0)
    chunks1, P1 = pad_split(idx1, n1)
    T0, T1 = P0 // TILE_N, P1 // TILE_N

    nc = _build_program(T0, T1)

    # --- weights (shared across cores) ---
    hc2 = h_child.reshape(n, 2 * H)
    cc2 = c_child.reshape(n, 2 * H)

    def bias_tile(v, m):
        # [m*128] bias vector -> [128, m] per-partition layout
        return np.ascontiguousarray(
            np.asarray(v, np.float32).reshape(-1)[: 128 * m].reshape(m, 128).T
        )

    def w_with_bias(W, b):
        # [XPAD, 768] = W^T with the iou bias as row 300, zero-padded to 384
        out = np.zeros((XPAD, 3 * H), dtype=NP_FP16)
        out[:X] = np.asarray(W, np.float32).T.astype(NP_FP16)
        out[X] = np.asarray(b, np.float32).reshape(-1).astype(NP_FP16)
        return out

    wmap = {
        "WnT": w_with_bias(W_iou, b_iou),
        "UnT": np.ascontiguousarray(np.asarray(U_iou, np.float32).T).astype(NP_FP16),
        "UfwT": np.ascontiguousarray(np.asarray(U_f_w, np.float32).T).astype(NP_FP16),
        "WsT": w_with_bias(W_iou_s, b_iou_s),
        "UsT": np.ascontiguousarray(np.asarray(U_iou_s, np.float32).T).astype(NP_FP16),
        "UfswT": np.ascontiguousarray(np.asarray(U_f_s_w, np.float32).T).astype(NP_FP16),
        "bias_fn": bias_tile(U_f_b, 4),
        "bias_fs": bias_tile(U_f_s_b, 2),
    }

    in_maps = []
    for i in range(CORES):
        I = np.concatenate([chunks0[i], chunks1[i]])
        m = dict(wmap)
        xTi = np.zeros((XPAD, len(I)), dtype=NP_FP16)
        xTi[:X] = x[I].T.astype(NP_FP16)
        xTi[X] = 1.0
        m["xT"] = xTi
        m["hT"] = hc2[I].T.astype(NP_FP16)
        m["cT"] = cc2[I].T.astype(NP_FP16)
        in_maps.append(m)

    res = bass_utils.run_bass_kernel_spmd(
        nc, in_maps, core_ids=list(range(CORES)), trace=TRACE
    )
    LAST_EXEC_NS = res.exec_time_ns

    # --- scatter back ---
    h_out = np.empty((n, H), dtype=np.float32)
    c_out = np.empty((n, H), dtype=np.float32)
    if n0:
        h0 = np.concatenate([res.results[i]["hOT"][:, :P0].T for i in range(CORES)])
        c0 = np.concatenate([res.results[i]["cOT"][:, :P0].T for i in range(CORES)])
        h_out[idx0] = h0[:n0].astype(np.float32)
        c_out[idx0] = c0[:n0].astype(np.float32)
    if n1:
        h1 = np.concatenate([res.results[i]["hOT"][:, P0:].T for i in range(CORES)])
        c1 = np.concatenate([res.results[i]["cOT"][:, P0:].T for i in range(CORES)])
        h_out[idx1] = h1[:n1].astype(np.float32)
        c_out[idx1] = c1[:n1].astype(np.float32)
    return h_out, c_out
